# revision 5
# baseline (speedup 1.0000x reference)
"""MoE layer (4 experts, top-2, capacity 10240) on 8 Trainium2 NeuronCores.

Strategy: the router + dispatch/combine index logic is deterministic integer
work computed on the host (f64 logits — verified to give routing identical to
any fp32 evaluation: min 2nd-vs-3rd logit gap is 4.5e-6 vs fp32 matmul noise
<2.3e-6, and all fp32 variants tested match f64 exactly). The heavy compute —
gelu-MLP over 4 experts x 10240 capacity slots (687 GFLOP) — runs on device,
expert-parallel: core c handles half h=c%2 of expert e=c//2 (5120 token slots).

Device kernel (identical SPMD program on all 8 cores):
  Y^T = W2^T @ gelu(W1^T @ X^T + b1) + b2
computed in bf16 with f32 PSUM accumulation, token tiles of 512, all weights
resident in SBUF. Layouts keep the contraction dim on partitions:
  mm1: psum[h*128.., tt] += w1[kc, :, h-chunk].T @ xT[kc, :, tt]   (kc: 8 D-chunks)
  mm2: psum[d*128.., tt] += w2[kh, :, d-chunk].T @ h_sb[kh][:, tt] (kh: 32 H-chunks)
"""

import numpy as np
import ml_dtypes

import concourse.mybir as mybir
import concourse.tile as tile
from concourse import bacc
from concourse import bass_utils

NUM_EXPERTS = 4
TOP_K = 2
B, S, D, H = 8, 4096, 1024, 4096
T = B * S                                   # 32768 tokens
CAPACITY = int(1.25 * S * B / NUM_EXPERTS)  # 10240
N_CORES = 8
TOK_PER_CORE = CAPACITY * NUM_EXPERTS // N_CORES  # 5120
TT = 512                                    # token tile
KC = D // 128                               # 8  (D chunks)
KH = H // 128                               # 32 (H chunks)

BF16 = ml_dtypes.bfloat16

_CACHE = {}

# test/profiling hooks: test.py sets RUN_KWARGS = {"trace": True} and reads
# LAST_RES (a BassKernelResults) for exec_time_ns. The graded path leaves
# these untouched.
RUN_KWARGS = {}
LAST_RES = None


def _build_nc(n_tok, act=None):
    """Build the SPMD Bass program for one core: dense FFN over n_tok tokens."""
    if act is None:
        act = mybir.ActivationFunctionType.Gelu
    n_tiles = n_tok // TT
    nc = bacc.Bacc(
        "TRN2",
        target_bir_lowering=False,
        debug=False,
        enable_asserts=False,
        num_devices=N_CORES,
    )
    xt = nc.dram_tensor("xt", (D, n_tok), mybir.dt.bfloat16, kind="ExternalInput").ap()
    w1 = nc.dram_tensor("w1", (D, H), mybir.dt.bfloat16, kind="ExternalInput").ap()
    b1 = nc.dram_tensor("b1", (128, KH), mybir.dt.float32, kind="ExternalInput").ap()
    w2 = nc.dram_tensor("w2", (H, D), mybir.dt.bfloat16, kind="ExternalInput").ap()
    b2 = nc.dram_tensor("b2", (128, KC), mybir.dt.float32, kind="ExternalInput").ap()
    yt = nc.dram_tensor("yt", (D, n_tok), mybir.dt.float32, kind="ExternalOutput").ap()

    xt_t = xt.rearrange("(kc p) n -> kc p n", p=128)   # [8, 128, n_tok]
    w1_t = w1.rearrange("(kc p) h -> kc p h", p=128)   # [8, 128, 4096]
    w2_t = w2.rearrange("(kh p) d -> kh p d", p=128)   # [32, 128, 1024]
    yt_t = yt.rearrange("(dc p) n -> dc p n", p=128)   # [8, 128, n_tok]

    with tile.TileContext(nc) as tc:
        with (
            tc.tile_pool(name="wpool", bufs=1) as wpool,
            tc.tile_pool(name="xpool", bufs=10) as xpool,
            tc.tile_pool(name="hpool", bufs=KH) as hpool,
            tc.tile_pool(name="ypool", bufs=1) as ypool,
            tc.tile_pool(name="p1", bufs=2, space="PSUM") as p1pool,
            tc.tile_pool(name="p2", bufs=2, space="PSUM") as p2pool,
        ):
            w1_sb = wpool.tile([128, KC, H], mybir.dt.bfloat16)
            w2_sb = wpool.tile([128, KH, D], mybir.dt.bfloat16)
            b1_sb = wpool.tile([128, KH], mybir.dt.float32)
            b2_sb = wpool.tile([128, KC], mybir.dt.float32)
            for kc in range(KC):
                nc.sync.dma_start(out=w1_sb[:, kc, :], in_=w1_t[kc])
            for kh in range(KH):
                nc.sync.dma_start(out=w2_sb[:, kh, :], in_=w2_t[kh])
            nc.sync.dma_start(out=b1_sb, in_=b1)
            nc.sync.dma_start(out=b2_sb, in_=b2)

            for t in range(n_tiles):
                tsl = slice(t * TT, (t + 1) * TT)
                x_ch = []
                for kc in range(KC):
                    xc = xpool.tile([128, TT], mybir.dt.bfloat16, tag="xc")
                    nc.sync.dma_start(out=xc, in_=xt_t[kc, :, tsl])
                    x_ch.append(xc)
                h_ch = []
                for h in range(KH):
                    ps = p1pool.tile([128, TT], mybir.dt.float32, tag="ps1")
                    for kc in range(KC):
                        nc.tensor.matmul(
                            ps,
                            w1_sb[:, kc, h * 128:(h + 1) * 128],
                            x_ch[kc],
                            start=(kc == 0),
                            stop=(kc == KC - 1),
                        )
                    hc = hpool.tile([128, TT], mybir.dt.bfloat16, tag="hc")
                    nc.scalar.activation(
                        out=hc,
                        in_=ps,
                        func=act,
                        bias=b1_sb[:, h:h + 1],
                        scale=1.0,
                    )
                    h_ch.append(hc)
                y_sb = ypool.tile([128, KC, TT], mybir.dt.float32, tag="y")
                for d in range(KC):
                    ps2 = p2pool.tile([128, TT], mybir.dt.float32, tag="ps2")
                    for kh in range(KH):
                        nc.tensor.matmul(
                            ps2,
                            w2_sb[:, kh, d * 128:(d + 1) * 128],
                            h_ch[kh],
                            start=(kh == 0),
                            stop=(kh == KH - 1),
                        )
                    nc.vector.tensor_scalar_add(
                        out=y_sb[:, d, :], in0=ps2, scalar1=b2_sb[:, d:d + 1]
                    )
                    nc.sync.dma_start(out=yt_t[d, :, tsl], in_=y_sb[:, d, :])
    nc.compile()
    return nc


def _get_nc(n_tok=TOK_PER_CORE):
    if n_tok not in _CACHE:
        _CACHE[n_tok] = _build_nc(n_tok)
    return _CACHE[n_tok]


def _route(xf, Wg, bg):
    """Replicates: logits = x@Wg+bg; top-2 (ties -> lower index); per-expert
    keep of the first CAPACITY routed tokens in flat order. f64 logits."""
    logits = xf.astype(np.float64) @ Wg.astype(np.float64) + bg.astype(np.float64)
    top2 = np.argsort(-logits, axis=1, kind="stable")[:, :TOP_K]  # [T, 2]
    toks = np.empty((NUM_EXPERTS, CAPACITY), np.int64)
    for e in range(NUM_EXPERTS):
        mask = (top2 == e).any(axis=1)
        keep = mask & (np.cumsum(mask) <= CAPACITY)
        tok = np.nonzero(keep)[0]
        toks[e] = np.concatenate([tok, np.full(CAPACITY - len(tok), T, np.int64)])
    return top2, toks


def kernel(x, Wg, bg, W1, b1, W2, b2):
    x = np.asarray(x, np.float32)
    Wg = np.asarray(Wg, np.float32)
    bg = np.asarray(bg, np.float32)
    W1 = np.asarray(W1, np.float32)
    b1 = np.asarray(b1, np.float32)
    W2 = np.asarray(W2, np.float32)
    b2 = np.asarray(b2, np.float32)

    xf = x.reshape(T, D)
    top2, toks = _route(xf, Wg, bg)

    in_maps = []
    for c in range(N_CORES):
        e, half = divmod(c, 2)
        rows = toks[e][half * TOK_PER_CORE:(half + 1) * TOK_PER_CORE]
        xe = xf[np.minimum(rows, T - 1)]  # [5120, D] (clamped, like reference)
        in_maps.append({
            "xt": np.ascontiguousarray(xe.T).astype(BF16),
            "w1": W1[e].astype(BF16),
            "b1": np.ascontiguousarray(b1[e].reshape(KH, 128).T),
            "w2": W2[e].astype(BF16),
            "b2": np.ascontiguousarray(b2[e].reshape(KC, 128).T),
        })

    nc = _get_nc()
    res = bass_utils.run_bass_kernel_spmd(
        nc, in_maps, core_ids=list(range(N_CORES)), **RUN_KWARGS
    )
    global LAST_RES
    LAST_RES = res

    out = np.zeros((T, D), np.float32)
    for e in range(NUM_EXPERTS):
        ye = np.concatenate(
            [np.asarray(res.results[2 * e + h]["yt"], np.float32).T for h in (0, 1)],
            axis=0,
        )  # [CAPACITY, D]
        tok = toks[e]
        valid = tok < T
        out[tok[valid]] += ye[valid]  # tok unique within an expert

    counts = np.bincount(top2.ravel(), minlength=NUM_EXPERTS).astype(np.float64)
    loss = np.float32(counts.std(ddof=1) / counts.mean())
    return out.reshape(B, S, D), loss


# revision 7
# speedup vs baseline: 1.0223x; 1.0223x over previous
"""MoE layer (4 experts, top-2, capacity 10240) on 8 Trainium2 NeuronCores.

Strategy: the router + dispatch/combine index logic is deterministic integer
work computed on the host (f64 logits — verified to give routing identical to
any fp32 evaluation: min 2nd-vs-3rd logit gap is 4.5e-6 vs fp32 matmul noise
<2.3e-6, and all fp32 variants tested match f64 exactly). The heavy compute —
gelu-MLP over 4 experts x 10240 capacity slots (687 GFLOP) — runs on device,
expert-parallel: core c handles half h=c%2 of expert e=c//2 (5120 token slots).

Device kernel (identical SPMD program on all 8 cores):
  Y^T = W2^T @ gelu(W1^T @ X^T + b1) + b2
computed in bf16 with f32 PSUM accumulation, token tiles of 512, all weights
resident in SBUF. Layouts keep the contraction dim on partitions:
  mm1: psum[h*128.., tt] += w1[kc, :, h-chunk].T @ xT[kc, :, tt]   (kc: 8 D-chunks)
  mm2: psum[d*128.., tt] += w2[kh, :, d-chunk].T @ h_sb[kh][:, tt] (kh: 32 H-chunks)
"""

import numpy as np
import ml_dtypes

import concourse.mybir as mybir
import concourse.tile as tile
from concourse import bacc
from concourse import bass_utils

NUM_EXPERTS = 4
TOP_K = 2
B, S, D, H = 8, 4096, 1024, 4096
T = B * S                                   # 32768 tokens
CAPACITY = int(1.25 * S * B / NUM_EXPERTS)  # 10240
N_CORES = 8
TOK_PER_CORE = CAPACITY * NUM_EXPERTS // N_CORES  # 5120
TT = 512                                    # token tile
KC = D // 128                               # 8  (D chunks)
KH = H // 128                               # 32 (H chunks)

BF16 = ml_dtypes.bfloat16

_CACHE = {}

# test/profiling hooks: test.py sets RUN_KWARGS = {"trace": True} and reads
# LAST_RES (a BassKernelResults) for exec_time_ns. The graded path leaves
# these untouched.
RUN_KWARGS = {}
LAST_RES = None


def _build_nc(n_tok, act=None):
    """Build the SPMD Bass program for one core: dense FFN over n_tok tokens."""
    if act is None:
        act = mybir.ActivationFunctionType.Gelu
    n_tiles = n_tok // TT
    nc = bacc.Bacc(
        "TRN2",
        target_bir_lowering=False,
        debug=False,
        enable_asserts=False,
        num_devices=N_CORES,
    )
    xt = nc.dram_tensor("xt", (D, n_tok), mybir.dt.bfloat16, kind="ExternalInput").ap()
    w1 = nc.dram_tensor("w1", (D, H), mybir.dt.bfloat16, kind="ExternalInput").ap()
    b1 = nc.dram_tensor("b1", (128, KH), mybir.dt.float32, kind="ExternalInput").ap()
    w2 = nc.dram_tensor("w2", (H, D), mybir.dt.bfloat16, kind="ExternalInput").ap()
    b2 = nc.dram_tensor("b2", (128, KC), mybir.dt.float32, kind="ExternalInput").ap()
    yt = nc.dram_tensor("yt", (D, n_tok), mybir.dt.float32, kind="ExternalOutput").ap()

    xt_t = xt.rearrange("(kc p) n -> kc p n", p=128)   # [8, 128, n_tok]
    w1_t = w1.rearrange("(kc p) h -> kc p h", p=128)   # [8, 128, 4096]
    w2_t = w2.rearrange("(kh p) d -> kh p d", p=128)   # [32, 128, 1024]
    yt_t = yt.rearrange("(dc p) n -> dc p n", p=128)   # [8, 128, n_tok]

    with tile.TileContext(nc) as tc:
        with (
            tc.tile_pool(name="wpool", bufs=1) as wpool,
            tc.tile_pool(name="xpool", bufs=12) as xpool,
            tc.tile_pool(name="hpool", bufs=KH) as hpool,
            tc.tile_pool(name="ypool", bufs=1) as ypool,
            tc.tile_pool(name="p1", bufs=2, space="PSUM") as p1pool,
            tc.tile_pool(name="p2", bufs=2, space="PSUM") as p2pool,
        ):
            w1_sb = wpool.tile([128, KC, H], mybir.dt.bfloat16)
            w2_sb = wpool.tile([128, KH, D], mybir.dt.bfloat16)
            b1_sb = wpool.tile([128, KH], mybir.dt.float32)
            b2_sb = wpool.tile([128, KC], mybir.dt.float32)

            def load_x_tile(t):
                tsl = slice(t * TT, (t + 1) * TT)
                ch = []
                for kc in range(KC):
                    xc = xpool.tile([128, TT], mybir.dt.bfloat16, tag="xc")
                    nc.sync.dma_start(out=xc, in_=xt_t[kc, :, tsl])
                    ch.append(xc)
                return ch

            # DMA issue order is the HWDGE queue order: x(t0) + w1 first so the
            # first matmul can start after ~9.4 MB instead of after all 17.9 MB
            # of weights; w2 streams in during mm1 of tile 0 (needed ~55 us in).
            x_next = load_x_tile(0)
            for kc in range(KC):
                nc.sync.dma_start(out=w1_sb[:, kc, :], in_=w1_t[kc])
            nc.sync.dma_start(out=b1_sb, in_=b1)
            for kh in range(KH):
                nc.sync.dma_start(out=w2_sb[:, kh, :], in_=w2_t[kh])
            nc.sync.dma_start(out=b2_sb, in_=b2)

            for t in range(n_tiles):
                tsl = slice(t * TT, (t + 1) * TT)
                x_ch = x_next
                if t + 1 < n_tiles:
                    x_next = load_x_tile(t + 1)
                h_ch = []
                for h in range(KH):
                    ps = p1pool.tile([128, TT], mybir.dt.float32, tag="ps1")
                    for kc in range(KC):
                        nc.tensor.matmul(
                            ps,
                            w1_sb[:, kc, h * 128:(h + 1) * 128],
                            x_ch[kc],
                            start=(kc == 0),
                            stop=(kc == KC - 1),
                        )
                    hc = hpool.tile([128, TT], mybir.dt.bfloat16, tag="hc")
                    nc.scalar.activation(
                        out=hc,
                        in_=ps,
                        func=act,
                        bias=b1_sb[:, h:h + 1],
                        scale=1.0,
                    )
                    h_ch.append(hc)
                y_sb = ypool.tile([128, KC, TT], mybir.dt.float32, tag="y")
                for d in range(KC):
                    ps2 = p2pool.tile([128, TT], mybir.dt.float32, tag="ps2")
                    for kh in range(KH):
                        nc.tensor.matmul(
                            ps2,
                            w2_sb[:, kh, d * 128:(d + 1) * 128],
                            h_ch[kh],
                            start=(kh == 0),
                            stop=(kh == KH - 1),
                        )
                    nc.vector.tensor_scalar_add(
                        out=y_sb[:, d, :], in0=ps2, scalar1=b2_sb[:, d:d + 1]
                    )
                    nc.sync.dma_start(out=yt_t[d, :, tsl], in_=y_sb[:, d, :])
    nc.compile()
    return nc


def _get_nc(n_tok=TOK_PER_CORE):
    if n_tok not in _CACHE:
        _CACHE[n_tok] = _build_nc(n_tok)
    return _CACHE[n_tok]


def _route(xf, Wg, bg):
    """Replicates: logits = x@Wg+bg; top-2 (ties -> lower index); per-expert
    keep of the first CAPACITY routed tokens in flat order. f64 logits."""
    logits = xf.astype(np.float64) @ Wg.astype(np.float64) + bg.astype(np.float64)
    top2 = np.argsort(-logits, axis=1, kind="stable")[:, :TOP_K]  # [T, 2]
    toks = np.empty((NUM_EXPERTS, CAPACITY), np.int64)
    for e in range(NUM_EXPERTS):
        mask = (top2 == e).any(axis=1)
        keep = mask & (np.cumsum(mask) <= CAPACITY)
        tok = np.nonzero(keep)[0]
        toks[e] = np.concatenate([tok, np.full(CAPACITY - len(tok), T, np.int64)])
    return top2, toks


def kernel(x, Wg, bg, W1, b1, W2, b2):
    x = np.asarray(x, np.float32)
    Wg = np.asarray(Wg, np.float32)
    bg = np.asarray(bg, np.float32)
    W1 = np.asarray(W1, np.float32)
    b1 = np.asarray(b1, np.float32)
    W2 = np.asarray(W2, np.float32)
    b2 = np.asarray(b2, np.float32)

    xf = x.reshape(T, D)
    top2, toks = _route(xf, Wg, bg)

    in_maps = []
    for c in range(N_CORES):
        e, half = divmod(c, 2)
        rows = toks[e][half * TOK_PER_CORE:(half + 1) * TOK_PER_CORE]
        xe = xf[np.minimum(rows, T - 1)]  # [5120, D] (clamped, like reference)
        in_maps.append({
            "xt": np.ascontiguousarray(xe.T).astype(BF16),
            "w1": W1[e].astype(BF16),
            "b1": np.ascontiguousarray(b1[e].reshape(KH, 128).T),
            "w2": W2[e].astype(BF16),
            "b2": np.ascontiguousarray(b2[e].reshape(KC, 128).T),
        })

    nc = _get_nc()
    res = bass_utils.run_bass_kernel_spmd(
        nc, in_maps, core_ids=list(range(N_CORES)), **RUN_KWARGS
    )
    global LAST_RES
    LAST_RES = res

    out = np.zeros((T, D), np.float32)
    for e in range(NUM_EXPERTS):
        ye = np.concatenate(
            [np.asarray(res.results[2 * e + h]["yt"], np.float32).T for h in (0, 1)],
            axis=0,
        )  # [CAPACITY, D]
        tok = toks[e]
        valid = tok < T
        out[tok[valid]] += ye[valid]  # tok unique within an expert

    counts = np.bincount(top2.ravel(), minlength=NUM_EXPERTS).astype(np.float64)
    loss = np.float32(counts.std(ddof=1) / counts.mean())
    return out.reshape(B, S, D), loss


# revision 12
# speedup vs baseline: 1.0238x; 1.0015x over previous
"""MoE layer (4 experts, top-2, capacity 10240) on 8 Trainium2 NeuronCores.

Strategy: the router + dispatch/combine index logic is deterministic integer
work computed on the host (f64 logits — verified to give routing identical to
any fp32 evaluation: min 2nd-vs-3rd logit gap is 4.5e-6 vs fp32 matmul noise
<2.3e-6, and all fp32 variants tested match f64 exactly). The heavy compute —
gelu-MLP over 4 experts x 10240 capacity slots (687 GFLOP) — runs on device,
expert-parallel: core c handles half h=c%2 of expert e=c//2 (5120 token slots).

Device kernel (identical SPMD program on all 8 cores):
  Y^T = W2^T @ gelu(W1^T @ X^T + b1) + b2
computed in bf16 with f32 PSUM accumulation, token tiles of 512, all weights
resident in SBUF. Layouts keep the contraction dim on partitions:
  mm1: psum[h*128.., tt] += w1[kc, :, h-chunk].T @ xT[kc, :, tt]   (kc: 8 D-chunks)
  mm2: psum[d*128.., tt] += w2[kh, :, d-chunk].T @ h_sb[kh][:, tt] (kh: 32 H-chunks)
"""

import numpy as np
import ml_dtypes

import concourse.mybir as mybir
import concourse.tile as tile
from concourse import bacc
from concourse import bass_utils

NUM_EXPERTS = 4
TOP_K = 2
B, S, D, H = 8, 4096, 1024, 4096
T = B * S                                   # 32768 tokens
CAPACITY = int(1.25 * S * B / NUM_EXPERTS)  # 10240
N_CORES = 8
TOK_PER_CORE = CAPACITY * NUM_EXPERTS // N_CORES  # 5120
TT = 512                                    # token tile
KC = D // 128                               # 8  (D chunks)
KH = H // 128                               # 32 (H chunks)

BF16 = ml_dtypes.bfloat16

_CACHE = {}

# test/profiling hooks: test.py sets RUN_KWARGS = {"trace": True} and reads
# LAST_RES (a BassKernelResults) for exec_time_ns. The graded path leaves
# these untouched.
RUN_KWARGS = {}
LAST_RES = None


def _build_nc(n_tok, act=None):
    """Build the SPMD Bass program for one core: dense FFN over n_tok tokens."""
    if act is None:
        act = mybir.ActivationFunctionType.Gelu
    n_tiles = n_tok // TT
    nc = bacc.Bacc(
        "TRN2",
        target_bir_lowering=False,
        debug=False,
        enable_asserts=False,
        num_devices=N_CORES,
    )
    xt = nc.dram_tensor("xt", (D, n_tok), mybir.dt.bfloat16, kind="ExternalInput").ap()
    w1 = nc.dram_tensor("w1", (D, H), mybir.dt.bfloat16, kind="ExternalInput").ap()
    b1 = nc.dram_tensor("b1", (128, KH), mybir.dt.float32, kind="ExternalInput").ap()
    w2 = nc.dram_tensor("w2", (H, D), mybir.dt.bfloat16, kind="ExternalInput").ap()
    b2 = nc.dram_tensor("b2", (128, KC), mybir.dt.float32, kind="ExternalInput").ap()
    yt = nc.dram_tensor("yt", (D, n_tok), mybir.dt.float32, kind="ExternalOutput").ap()

    xt_t = xt.rearrange("(kc p) n -> kc p n", p=128)   # [8, 128, n_tok]
    w1_t = w1.rearrange("(kc p) h -> kc p h", p=128)   # [8, 128, 4096]
    w2_t = w2.rearrange("(kh p) d -> kh p d", p=128)   # [32, 128, 1024]
    yt_t = yt.rearrange("(dc p) n -> dc p n", p=128)   # [8, 128, n_tok]

    with tile.TileContext(nc) as tc:
        with (
            tc.tile_pool(name="wpool", bufs=1) as wpool,
            tc.tile_pool(name="xpool", bufs=12) as xpool,
            tc.tile_pool(name="hpool", bufs=KH) as hpool,
            tc.tile_pool(name="ypool", bufs=1) as ypool,
            tc.tile_pool(name="pp", bufs=8, space="PSUM") as ppool,
        ):
            w1_sb = wpool.tile([128, KC, H], mybir.dt.bfloat16)
            w2_sb = wpool.tile([128, KH, D], mybir.dt.bfloat16)
            b1_sb = wpool.tile([128, KH], mybir.dt.float32)
            b2_sb = wpool.tile([128, KC], mybir.dt.float32)

            def load_x_tile(t):
                tsl = slice(t * TT, (t + 1) * TT)
                ch = []
                for kc in range(KC):
                    xc = xpool.tile([128, TT], mybir.dt.bfloat16, tag="xc")
                    nc.sync.dma_start(out=xc, in_=xt_t[kc, :, tsl])
                    ch.append(xc)
                return ch

            # DMA issue order is the HWDGE queue order. w1/x0 chunk pairs go
            # first (the first-tile matmuls consume each (w1[kc], x0[kc]) pair
            # as it lands); w2 streams in during mm1 of tile 0 (it is not
            # needed until mm2 of tile 0, ~80 us in).
            x_next = []
            for kc in range(KC):
                nc.sync.dma_start(out=w1_sb[:, kc, :], in_=w1_t[kc])
                xc = xpool.tile([128, TT], mybir.dt.bfloat16, tag="xc")
                nc.sync.dma_start(out=xc, in_=xt_t[kc, :, 0:TT])
                x_next.append(xc)
            nc.sync.dma_start(out=b1_sb, in_=b1)
            for kh in range(KH):
                nc.sync.dma_start(out=w2_sb[:, kh, :], in_=w2_t[kh])
            nc.sync.dma_start(out=b2_sb, in_=b2)

            for t in range(n_tiles):
                tsl = slice(t * TT, (t + 1) * TT)
                x_ch = x_next
                if t + 1 < n_tiles:
                    x_next = load_x_tile(t + 1)
                def gelu_drain(ps, h):
                    hc = hpool.tile([128, TT], mybir.dt.bfloat16, tag="hc")
                    nc.scalar.activation(
                        out=hc, in_=ps, func=act, bias=b1_sb[:, h:h + 1], scale=1.0
                    )
                    return hc

                h_ch = []
                if t == 0:
                    # First tile: kc-outer over batches of 8 h-chunks (all 8
                    # PSUM banks) so the PE consumes each w1 chunk as its DMA
                    # lands instead of idling until w1 is fully resident.
                    for g in range(0, KH, 8):
                        pss = [
                            ppool.tile([128, TT], mybir.dt.float32, tag="ps",
                                       name=f"ps_t0_g{g}_{j}")
                            for j in range(8)
                        ]
                        for kc in range(KC):
                            for j in range(8):
                                h = g + j
                                nc.tensor.matmul(
                                    pss[j],
                                    w1_sb[:, kc, h * 128:(h + 1) * 128],
                                    x_ch[kc],
                                    start=(kc == 0),
                                    stop=(kc == KC - 1),
                                )
                        for j in range(8):
                            h_ch.append(gelu_drain(pss[j], g + j))
                else:
                    for h in range(KH):
                        ps = ppool.tile([128, TT], mybir.dt.float32, tag="ps")
                        for kc in range(KC):
                            nc.tensor.matmul(
                                ps,
                                w1_sb[:, kc, h * 128:(h + 1) * 128],
                                x_ch[kc],
                                start=(kc == 0),
                                stop=(kc == KC - 1),
                            )
                        h_ch.append(gelu_drain(ps, h))
                y_sb = ypool.tile([128, KC, TT], mybir.dt.float32, tag="y")
                for d in range(KC):
                    ps2 = ppool.tile([128, TT], mybir.dt.float32, tag="ps")
                    for kh in range(KH):
                        nc.tensor.matmul(
                            ps2,
                            w2_sb[:, kh, d * 128:(d + 1) * 128],
                            h_ch[kh],
                            start=(kh == 0),
                            stop=(kh == KH - 1),
                        )
                    nc.vector.tensor_scalar_add(
                        out=y_sb[:, d, :], in0=ps2, scalar1=b2_sb[:, d:d + 1]
                    )
                    nc.sync.dma_start(out=yt_t[d, :, tsl], in_=y_sb[:, d, :])
    nc.compile()
    return nc


def _get_nc(n_tok=TOK_PER_CORE):
    if n_tok not in _CACHE:
        _CACHE[n_tok] = _build_nc(n_tok)
    return _CACHE[n_tok]


def _route(xf, Wg, bg):
    """Replicates: logits = x@Wg+bg; top-2 (ties -> lower index); per-expert
    keep of the first CAPACITY routed tokens in flat order. f64 logits."""
    logits = xf.astype(np.float64) @ Wg.astype(np.float64) + bg.astype(np.float64)
    top2 = np.argsort(-logits, axis=1, kind="stable")[:, :TOP_K]  # [T, 2]
    toks = np.empty((NUM_EXPERTS, CAPACITY), np.int64)
    for e in range(NUM_EXPERTS):
        mask = (top2 == e).any(axis=1)
        keep = mask & (np.cumsum(mask) <= CAPACITY)
        tok = np.nonzero(keep)[0]
        toks[e] = np.concatenate([tok, np.full(CAPACITY - len(tok), T, np.int64)])
    return top2, toks


def kernel(x, Wg, bg, W1, b1, W2, b2):
    x = np.asarray(x, np.float32)
    Wg = np.asarray(Wg, np.float32)
    bg = np.asarray(bg, np.float32)
    W1 = np.asarray(W1, np.float32)
    b1 = np.asarray(b1, np.float32)
    W2 = np.asarray(W2, np.float32)
    b2 = np.asarray(b2, np.float32)

    xf = x.reshape(T, D)
    top2, toks = _route(xf, Wg, bg)

    in_maps = []
    for c in range(N_CORES):
        e, half = divmod(c, 2)
        rows = toks[e][half * TOK_PER_CORE:(half + 1) * TOK_PER_CORE]
        xe = xf[np.minimum(rows, T - 1)]  # [5120, D] (clamped, like reference)
        in_maps.append({
            "xt": np.ascontiguousarray(xe.T).astype(BF16),
            "w1": W1[e].astype(BF16),
            "b1": np.ascontiguousarray(b1[e].reshape(KH, 128).T),
            "w2": W2[e].astype(BF16),
            "b2": np.ascontiguousarray(b2[e].reshape(KC, 128).T),
        })

    nc = _get_nc()
    res = bass_utils.run_bass_kernel_spmd(
        nc, in_maps, core_ids=list(range(N_CORES)), **RUN_KWARGS
    )
    global LAST_RES
    LAST_RES = res

    out = np.zeros((T, D), np.float32)
    for e in range(NUM_EXPERTS):
        ye = np.concatenate(
            [np.asarray(res.results[2 * e + h]["yt"], np.float32).T for h in (0, 1)],
            axis=0,
        )  # [CAPACITY, D]
        tok = toks[e]
        valid = tok < T
        out[tok[valid]] += ye[valid]  # tok unique within an expert

    counts = np.bincount(top2.ravel(), minlength=NUM_EXPERTS).astype(np.float64)
    loss = np.float32(counts.std(ddof=1) / counts.mean())
    return out.reshape(B, S, D), loss


# revision 13
# speedup vs baseline: 1.0367x; 1.0126x over previous
"""MoE layer (4 experts, top-2, capacity 10240) on 8 Trainium2 NeuronCores.

Strategy: the router + dispatch/combine index logic is deterministic integer
work computed on the host (f64 logits — verified to give routing identical to
any fp32 evaluation: min 2nd-vs-3rd logit gap is 4.5e-6 vs fp32 matmul noise
<2.3e-6, and all fp32 variants tested match f64 exactly). The heavy compute —
gelu-MLP over 4 experts x 10240 capacity slots (687 GFLOP) — runs on device,
expert-parallel: core c handles half h=c%2 of expert e=c//2 (5120 token slots).

Device kernel (identical SPMD program on all 8 cores):
  Y^T = W2^T @ gelu(W1^T @ X^T + b1) + b2
computed in bf16 with f32 PSUM accumulation, token tiles of 512, all weights
resident in SBUF. Layouts keep the contraction dim on partitions:
  mm1: psum[h*128.., tt] += w1[kc, :, h-chunk].T @ xT[kc, :, tt]   (kc: 8 D-chunks)
  mm2: psum[d*128.., tt] += w2[kh, :, d-chunk].T @ h_sb[kh][:, tt] (kh: 32 H-chunks)
"""

import numpy as np
import ml_dtypes

import concourse.mybir as mybir
import concourse.tile as tile
from concourse import bacc
from concourse import bass_utils

NUM_EXPERTS = 4
TOP_K = 2
B, S, D, H = 8, 4096, 1024, 4096
T = B * S                                   # 32768 tokens
CAPACITY = int(1.25 * S * B / NUM_EXPERTS)  # 10240
N_CORES = 8
TOK_PER_CORE = CAPACITY * NUM_EXPERTS // N_CORES  # 5120
TT = 512                                    # token tile
KC = D // 128                               # 8  (D chunks)
KH = H // 128                               # 32 (H chunks)

BF16 = ml_dtypes.bfloat16

_CACHE = {}

# test/profiling hooks: test.py sets RUN_KWARGS = {"trace": True} and reads
# LAST_RES (a BassKernelResults) for exec_time_ns. The graded path leaves
# these untouched.
RUN_KWARGS = {}
LAST_RES = None


def _build_nc(n_tok, act=None):
    """Build the SPMD Bass program for one core: dense FFN over n_tok tokens."""
    if act is None:
        act = mybir.ActivationFunctionType.Gelu
    n_tiles = n_tok // TT
    nc = bacc.Bacc(
        "TRN2",
        target_bir_lowering=False,
        debug=False,
        enable_asserts=False,
        num_devices=N_CORES,
    )
    xt = nc.dram_tensor("xt", (D, n_tok), mybir.dt.bfloat16, kind="ExternalInput").ap()
    w1 = nc.dram_tensor("w1", (D, H), mybir.dt.bfloat16, kind="ExternalInput").ap()
    b1 = nc.dram_tensor("b1", (128, KH), mybir.dt.float32, kind="ExternalInput").ap()
    w2 = nc.dram_tensor("w2", (H, D), mybir.dt.bfloat16, kind="ExternalInput").ap()
    b2 = nc.dram_tensor("b2", (128, KC), mybir.dt.float32, kind="ExternalInput").ap()
    yt = nc.dram_tensor("yt", (D, n_tok), mybir.dt.float32, kind="ExternalOutput").ap()

    xt_t = xt.rearrange("(kc p) n -> kc p n", p=128)   # [8, 128, n_tok]
    w1_t = w1.rearrange("(kc p) h -> kc p h", p=128)   # [8, 128, 4096]
    w2_t = w2.rearrange("(kh p) d -> kh p d", p=128)   # [32, 128, 1024]
    yt_t = yt.rearrange("(dc p) n -> dc p n", p=128)   # [8, 128, n_tok]

    with tile.TileContext(nc) as tc:
        with (
            tc.tile_pool(name="wpool", bufs=1) as wpool,
            tc.tile_pool(name="xpool", bufs=12) as xpool,
            tc.tile_pool(name="hpool", bufs=KH) as hpool,
            tc.tile_pool(name="ypool", bufs=1) as ypool,
            tc.tile_pool(name="pp", bufs=8, space="PSUM") as ppool,
        ):
            w1_sb = wpool.tile([128, KC, H], mybir.dt.bfloat16)
            w2_sb = wpool.tile([128, KH, D], mybir.dt.bfloat16)
            b1_sb = wpool.tile([128, KH], mybir.dt.float32)
            b2_sb = wpool.tile([128, KC], mybir.dt.float32)

            def load_x_tile(t):
                tsl = slice(t * TT, (t + 1) * TT)
                ch = []
                for kc in range(KC):
                    xc = xpool.tile([128, TT], mybir.dt.bfloat16, tag="xc")
                    nc.sync.dma_start(out=xc, in_=xt_t[kc, :, tsl])
                    ch.append(xc)
                return ch

            # PE warmup: the HAM clock gate keeps the PE at 1.2 GHz until it
            # has been busy ~3.4 us, and re-throttles across idle windows.
            # Matmul a zeroed scratch tile (result never read) from ~2 us so
            # the PE is warm when the first real matmul issues at ~12 us and
            # stays warm through the DMA-paced first-tile phase.
            wu = wpool.tile([128, TT], mybir.dt.bfloat16, name="warmup_x")
            nc.vector.memset(wu, 0.0)
            wps = ppool.tile([128, TT], mybir.dt.float32, name="warmup_ps", tag="ps")
            for _ in range(32):
                nc.tensor.matmul(wps, wu[:, :128], wu, start=True, stop=True)

            # DMA issue order is the HWDGE queue order. w1/x0 chunk pairs go
            # first (the first-tile matmuls consume each (w1[kc], x0[kc]) pair
            # as it lands); w2 streams in during mm1 of tile 0 (it is not
            # needed until mm2 of tile 0, ~80 us in).
            x_next = []
            for kc in range(KC):
                nc.sync.dma_start(out=w1_sb[:, kc, :], in_=w1_t[kc])
                xc = xpool.tile([128, TT], mybir.dt.bfloat16, tag="xc")
                nc.sync.dma_start(out=xc, in_=xt_t[kc, :, 0:TT])
                x_next.append(xc)
            nc.sync.dma_start(out=b1_sb, in_=b1)
            for kh in range(KH):
                nc.sync.dma_start(out=w2_sb[:, kh, :], in_=w2_t[kh])
            nc.sync.dma_start(out=b2_sb, in_=b2)

            for t in range(n_tiles):
                tsl = slice(t * TT, (t + 1) * TT)
                x_ch = x_next
                if t + 1 < n_tiles:
                    x_next = load_x_tile(t + 1)
                def gelu_drain(ps, h):
                    hc = hpool.tile([128, TT], mybir.dt.bfloat16, tag="hc")
                    nc.scalar.activation(
                        out=hc, in_=ps, func=act, bias=b1_sb[:, h:h + 1], scale=1.0
                    )
                    return hc

                h_ch = []
                if t == 0:
                    # First tile: kc-outer over batches of 8 h-chunks (all 8
                    # PSUM banks) so the PE consumes each w1 chunk as its DMA
                    # lands instead of idling until w1 is fully resident.
                    for g in range(0, KH, 8):
                        pss = [
                            ppool.tile([128, TT], mybir.dt.float32, tag="ps",
                                       name=f"ps_t0_g{g}_{j}")
                            for j in range(8)
                        ]
                        for kc in range(KC):
                            for j in range(8):
                                h = g + j
                                nc.tensor.matmul(
                                    pss[j],
                                    w1_sb[:, kc, h * 128:(h + 1) * 128],
                                    x_ch[kc],
                                    start=(kc == 0),
                                    stop=(kc == KC - 1),
                                )
                        for j in range(8):
                            h_ch.append(gelu_drain(pss[j], g + j))
                else:
                    for h in range(KH):
                        ps = ppool.tile([128, TT], mybir.dt.float32, tag="ps")
                        for kc in range(KC):
                            nc.tensor.matmul(
                                ps,
                                w1_sb[:, kc, h * 128:(h + 1) * 128],
                                x_ch[kc],
                                start=(kc == 0),
                                stop=(kc == KC - 1),
                            )
                        h_ch.append(gelu_drain(ps, h))
                y_sb = ypool.tile([128, KC, TT], mybir.dt.float32, tag="y")
                for d in range(KC):
                    ps2 = ppool.tile([128, TT], mybir.dt.float32, tag="ps")
                    for kh in range(KH):
                        nc.tensor.matmul(
                            ps2,
                            w2_sb[:, kh, d * 128:(d + 1) * 128],
                            h_ch[kh],
                            start=(kh == 0),
                            stop=(kh == KH - 1),
                        )
                    nc.vector.tensor_scalar_add(
                        out=y_sb[:, d, :], in0=ps2, scalar1=b2_sb[:, d:d + 1]
                    )
                    nc.sync.dma_start(out=yt_t[d, :, tsl], in_=y_sb[:, d, :])
    nc.compile()
    return nc


def _get_nc(n_tok=TOK_PER_CORE):
    if n_tok not in _CACHE:
        _CACHE[n_tok] = _build_nc(n_tok)
    return _CACHE[n_tok]


def _route(xf, Wg, bg):
    """Replicates: logits = x@Wg+bg; top-2 (ties -> lower index); per-expert
    keep of the first CAPACITY routed tokens in flat order. f64 logits."""
    logits = xf.astype(np.float64) @ Wg.astype(np.float64) + bg.astype(np.float64)
    top2 = np.argsort(-logits, axis=1, kind="stable")[:, :TOP_K]  # [T, 2]
    toks = np.empty((NUM_EXPERTS, CAPACITY), np.int64)
    for e in range(NUM_EXPERTS):
        mask = (top2 == e).any(axis=1)
        keep = mask & (np.cumsum(mask) <= CAPACITY)
        tok = np.nonzero(keep)[0]
        toks[e] = np.concatenate([tok, np.full(CAPACITY - len(tok), T, np.int64)])
    return top2, toks


def kernel(x, Wg, bg, W1, b1, W2, b2):
    x = np.asarray(x, np.float32)
    Wg = np.asarray(Wg, np.float32)
    bg = np.asarray(bg, np.float32)
    W1 = np.asarray(W1, np.float32)
    b1 = np.asarray(b1, np.float32)
    W2 = np.asarray(W2, np.float32)
    b2 = np.asarray(b2, np.float32)

    xf = x.reshape(T, D)
    top2, toks = _route(xf, Wg, bg)

    in_maps = []
    for c in range(N_CORES):
        e, half = divmod(c, 2)
        rows = toks[e][half * TOK_PER_CORE:(half + 1) * TOK_PER_CORE]
        xe = xf[np.minimum(rows, T - 1)]  # [5120, D] (clamped, like reference)
        in_maps.append({
            "xt": np.ascontiguousarray(xe.T).astype(BF16),
            "w1": W1[e].astype(BF16),
            "b1": np.ascontiguousarray(b1[e].reshape(KH, 128).T),
            "w2": W2[e].astype(BF16),
            "b2": np.ascontiguousarray(b2[e].reshape(KC, 128).T),
        })

    nc = _get_nc()
    res = bass_utils.run_bass_kernel_spmd(
        nc, in_maps, core_ids=list(range(N_CORES)), **RUN_KWARGS
    )
    global LAST_RES
    LAST_RES = res

    out = np.zeros((T, D), np.float32)
    for e in range(NUM_EXPERTS):
        ye = np.concatenate(
            [np.asarray(res.results[2 * e + h]["yt"], np.float32).T for h in (0, 1)],
            axis=0,
        )  # [CAPACITY, D]
        tok = toks[e]
        valid = tok < T
        out[tok[valid]] += ye[valid]  # tok unique within an expert

    counts = np.bincount(top2.ravel(), minlength=NUM_EXPERTS).astype(np.float64)
    loss = np.float32(counts.std(ddof=1) / counts.mean())
    return out.reshape(B, S, D), loss
